# revision 4
# baseline (speedup 1.0000x reference)
"""Trainium2 Bass kernel for nn_Loss_20933670601009 (gathered-prob NLL loss).

Strategy: the loss touches 3 elements per (l, b) position (one gathered prob
from each of rule/token/reference tables). Indirect element-DMA maxes out at
128 offsets per ~1us SWDGE instruction (12 needed), so instead we use
dma_gather (Q7 mlp library): each of 3 calls gathers ALL its chunks in one
instruction — the 256B-aligned chunk containing each needed element — and a
host-prebuilt one-hot mask extracts the element on the DVE:

  call A (elem=64 f32):  512 rule chunks + 512 ref chunks   (1024 idxs)
  call B1/B2 (elem=256): 256+256 token chunks               (int16 idx limit)

Host-side index preprocessing folds validity (gt=-1) and the batch mask into
the chunk indices by redirecting to appended const rows (1.0 / 0.0), so the
device applies no mask at all: masked positions contribute log(1.0)=0.

Device tail: E = chunks * onehots; per-position sums via 3 segmented
reduces + 2 adds; fused activation Ln(s+eps) with free-axis accumulate;
PE matmul against -1/B weights for the partition sum.

Sharding: data-parallel over L_a (128 rows -> 16 x 8 cores, 512 positions
per core). Host sums the 8 per-core scalars.
"""

import os
import sys

import numpy as np

for _p in ("/opt/trn_rl_repo", "/root/.axon_site/_ro/trn_rl_repo"):
    if os.path.isdir(_p) and _p not in sys.path:
        sys.path.insert(0, _p)

L_A, B = 128, 32
V_RULE, V_TOK, V_REF = 2048, 32000, 512
EPS = 1e-07
N_CORES = 8
L_SH = L_A // N_CORES            # 16 sequence rows per core
NPOS = L_SH * B                  # 512 positions per core
P = 128
J = NPOS // P                    # 4 positions per partition

# table A: rule chunks (64 f32) then ref chunks then [1,0..] and zero rows
RULE_ROWS = NPOS * (V_RULE // 64)          # 16384
REF_ROWS = NPOS * (V_REF // 64)            # 4096
ONE_A = RULE_ROWS + REF_ROWS               # 20480: [1.0, 0 x63]
ZERO_A = ONE_A + 1                         # 20481: zeros
A_ROWS = ONE_A + 2
# tables B1/B2: token chunks (256 f32) for 256 positions each + const rows
TOK_HALF = NPOS // 2                       # 256 positions
TOK_ROWS = TOK_HALF * (V_TOK // 256)       # 32000
ONE_B = TOK_ROWS                           # 32000 (unused)
ZERO_B = TOK_ROWS + 1                      # 32001: zeros
B_ROWS = TOK_ROWS + 2

QUEUES = (0, 1, 2)                         # SWDGE queues for A, B1, B2

_CACHE = {}


def _build():
    """Build + compile the per-core Bass module (same NEFF on all 8 cores)."""
    import concourse.bacc as bacc
    import concourse.mybir as mybir
    import concourse.tile as tile
    from concourse.library_config import mlp

    f32 = mybir.dt.float32
    i16 = mybir.dt.int16

    nc = bacc.Bacc(
        "TRN2",
        target_bir_lowering=False,
        debug=False,
        enable_asserts=False,
        num_devices=N_CORES,
        num_swdge_queues=max(QUEUES) + 1,
    )

    # meta (int16 [128, 100]): 0:64 idxA wrap, 64:80 idxB1, 80:96 idxB2,
    # 96:98 -1/B f32 bits, 98:100 eps f32 bits. idx wrap layout: idx k ->
    # partition k%16, col k//16, replicated across the 8 groups of 16.
    meta_d = nc.dram_tensor("meta", [P, 100], i16, kind="ExternalInput").ap()
    mask_d = nc.dram_tensor("maskm", [P, 1536], f32, kind="ExternalInput").ap()
    ta_d = nc.dram_tensor("ta", [A_ROWS, 64], f32, kind="ExternalInput").ap()
    tb1_d = nc.dram_tensor("tb1", [B_ROWS, 256], f32, kind="ExternalInput").ap()
    tb2_d = nc.dram_tensor("tb2", [B_ROWS, 256], f32, kind="ExternalInput").ap()
    out_d = nc.dram_tensor("out", [1, 1], f32, kind="ExternalOutput").ap()

    with tile.TileContext(nc) as tc:
        with (
            tc.tile_pool(name="sb", bufs=1) as pool,
            tc.tile_pool(name="ps", bufs=1, space="PSUM") as psum,
        ):
            nc.gpsimd.load_library(mlp)
            meta = pool.tile([P, 100], i16)
            nc.sync.dma_start(out=meta[:], in_=meta_d[:])
            mask = pool.tile([P, 1536], f32)
            nc.sync.dma_start(out=mask[:], in_=mask_d[:])
            negw = meta[:, 96:98].bitcast(f32)
            epsb = meta[:, 98:100].bitcast(f32)

            t = pool.tile([P, 1536], f32)
            # layout of t (position q at partition q%128, j=q//128):
            #   cols    0:256  rule chunks  [128, 4, 64]
            #   cols  256:512  ref chunks   [128, 4, 64]
            #   cols 512:1536  token chunks [128, 4, 256]
            nc.gpsimd.dma_gather(
                out_ap=t[:, 0:512].rearrange("p (g e) -> p g e", e=64),
                in_ap=ta_d[:],
                idxs_ap=meta[:, 0:64],
                num_idxs=1024, num_idxs_reg=1024, elem_size=64,
                queue_num=QUEUES[0],
            )
            nc.gpsimd.dma_gather(
                out_ap=t[:, 512:1024].rearrange("p (g e) -> p g e", e=256),
                in_ap=tb1_d[:],
                idxs_ap=meta[:, 64:80],
                num_idxs=256, num_idxs_reg=256, elem_size=256,
                queue_num=QUEUES[1],
            )
            nc.gpsimd.dma_gather(
                out_ap=t[:, 1024:1536].rearrange("p (g e) -> p g e", e=256),
                in_ap=tb2_d[:],
                idxs_ap=meta[:, 80:96],
                num_idxs=256, num_idxs_reg=256, elem_size=256,
                queue_num=QUEUES[2],
            )

            # one-hot extraction (in place), then per-position sums
            nc.vector.tensor_mul(out=t[:, 0:512], in0=t[:, 0:512], in1=mask[:, 0:512])
            nc.vector.tensor_mul(
                out=t[:, 512:1536], in0=t[:, 512:1536], in1=mask[:, 512:1536]
            )
            r1 = pool.tile([P, J], f32)
            r2 = pool.tile([P, J], f32)
            r3 = pool.tile([P, J], f32)
            X = mybir.AxisListType.X
            nc.vector.reduce_sum(
                out=r1[:], in_=t[:, 0:256].rearrange("p (j e) -> p j e", e=64), axis=X)
            nc.vector.reduce_sum(
                out=r2[:], in_=t[:, 256:512].rearrange("p (j e) -> p j e", e=64), axis=X)
            nc.vector.reduce_sum(
                out=r3[:], in_=t[:, 512:1536].rearrange("p (j e) -> p j e", e=256), axis=X)
            s = pool.tile([P, J], f32)
            nc.vector.tensor_add(out=s[:], in0=r1[:], in1=r2[:])
            nc.vector.tensor_add(out=s[:], in0=s[:], in1=r3[:])

            # ln = Ln(s + eps); rs[p] = sum_j ln[p, j]
            # (log(x+eps) ~ torch's x + (x<eps)*eps to ~1e-7 absolute)
            ln = pool.tile([P, J], f32)
            rs = pool.tile([P, 1], f32)
            nc.scalar.activation(
                out=ln[:], in_=s[:], func=mybir.ActivationFunctionType.Ln,
                bias=epsb, accum_out=rs[:],
            )

            # partition reduction via PE; weight -1/B folds negation + mean
            acc = psum.tile([1, 1], f32)
            nc.tensor.matmul(out=acc[:], lhsT=rs[:], rhs=negw, start=True, stop=True)
            res = pool.tile([1, 1], f32)
            nc.scalar.copy(out=res[:], in_=acc[:])
            nc.sync.dma_start(out=out_d[:], in_=res[:])

    nc.compile()
    return nc


def get_nc():
    if "nc" not in _CACHE:
        _CACHE["nc"] = _build()
    return _CACHE["nc"]


def _wrap_idxs(a):
    """idx k -> partition k%16, col k//16, replicated across 8 groups."""
    w = a.reshape(-1, 16).T
    return np.tile(w, (8, 1))


def make_in_maps(rule_probs, token_probs, reference_probs, ground_truth_actions, mask):
    """Shard the full inputs into 8 per-core input maps."""
    rule_probs = np.ascontiguousarray(np.asarray(rule_probs, dtype=np.float32))
    token_probs = np.ascontiguousarray(np.asarray(token_probs, dtype=np.float32))
    reference_probs = np.ascontiguousarray(np.asarray(reference_probs, dtype=np.float32))
    gt = np.asarray(ground_truth_actions, dtype=np.int32)
    mask_in = np.asarray(mask, dtype=np.int32)

    q = np.arange(NPOS, dtype=np.int64)
    one64 = np.zeros(64, np.float32); one64[0] = 1.0
    zero64 = np.zeros(64, np.float32)
    one256 = np.zeros(256, np.float32); one256[0] = 1.0
    zero256 = np.zeros(256, np.float32)
    scal = np.empty(2, np.float32)
    scal[0] = -1.0 / B
    scal[1] = EPS
    scal16 = scal.view(np.int16)

    in_maps = []
    for i in range(N_CORES):
        lo, hi = i * L_SH, (i + 1) * L_SH
        gt_sh = gt[lo:hi].reshape(NPOS, 3).astype(np.int64)
        m = mask_in[lo:hi].reshape(NPOS) != 0
        gr, gtk, gf = gt_sh[:, 0], gt_sh[:, 1], gt_sh[:, 2]

        vr = (gr >= 0) & m
        idx_r = np.where(vr, q * 32 + np.maximum(gr, 0) // 64,
                         np.where(m, ZERO_A, ONE_A))
        pos_r = np.where(vr, np.maximum(gr, 0) % 64, 0)

        vf = (gf >= 0) & m
        idx_f = np.where(vf, RULE_ROWS + q * 8 + np.maximum(gf, 0) // 64, ZERO_A)
        pos_f = np.where(vf, np.maximum(gf, 0) % 64, 0)

        vt = (gtk >= 0) & m
        idx_t = np.where(vt, (q % TOK_HALF) * (V_TOK // 256)
                         + np.maximum(gtk, 0) // 256, ZERO_B)
        pos_t = np.where(vt, np.maximum(gtk, 0) % 256, 0)

        meta = np.empty((P, 100), np.int16)
        meta[:, 0:64] = _wrap_idxs(
            np.concatenate([idx_r, idx_f]).astype(np.int16))
        meta[:, 64:80] = _wrap_idxs(idx_t[0:TOK_HALF].astype(np.int16))
        meta[:, 80:96] = _wrap_idxs(idx_t[TOK_HALF:].astype(np.int16))
        meta[:, 96:100] = scal16

        p_of = (q % P).astype(np.int64)
        j_of = (q // P).astype(np.int64)
        mk = np.zeros((P, 1536), np.float32)
        mk[p_of, 64 * j_of + pos_r] = 1.0
        mk[p_of, 256 + 64 * j_of + pos_f] = 1.0
        mk[p_of, 512 + 256 * j_of + pos_t] = 1.0

        ta = np.concatenate([
            rule_probs[lo:hi].reshape(-1),
            reference_probs[lo:hi].reshape(-1),
            one64, zero64,
        ]).reshape(A_ROWS, 64)
        tok = token_probs[lo:hi].reshape(NPOS, V_TOK)
        tb1 = np.concatenate([
            tok[0:TOK_HALF].reshape(-1), one256, zero256]).reshape(B_ROWS, 256)
        tb2 = np.concatenate([
            tok[TOK_HALF:].reshape(-1), one256, zero256]).reshape(B_ROWS, 256)

        in_maps.append({
            "meta": meta, "maskm": mk, "ta": ta, "tb1": tb1, "tb2": tb2,
        })
    return in_maps


def run(inputs, trace=False, trace_cores=None):
    """Run on the 8 NeuronCores; returns (scalar ndarray, BassKernelResults)."""
    from concourse.bass_utils import run_bass_kernel_spmd

    nc = get_nc()
    in_maps = make_in_maps(**inputs)
    res = run_bass_kernel_spmd(
        nc,
        in_maps,
        core_ids=list(range(N_CORES)),
        trace=trace,
        trace_cores=trace_cores,
    )
    total = np.float64(0.0)
    for r in res.results:
        total += np.float64(r["out"].reshape(())[()])
    return np.asarray(total, dtype=np.float32), res


def kernel(**inputs) -> np.ndarray:
    out, _ = run(inputs)
    return out
